# revision 11
# baseline (speedup 1.0000x reference)
"""AttentionPooling GNN kernel for 8 Trainium2 NeuronCores.

Strategy
--------
Graph-parallel sharding: 128 graphs -> 16 graphs per core; each core gets its
graphs' nodes and (re-grouped) edges.  Host does index preprocessing only
(edge permutation by source node, degree counts, weight folding); all FLOPs
on the edge/node payloads run on device.

Device algorithm (per core):
 1. Edges arrive fp8, pre-scaled by 1/deg(src), sorted by source node and
    padded so each node's run occupies whole 4-edge slots inside a single
    128-edge chunk.  A shared triangular stationary (TriU4, fp8) turns chunk
    matmuls into slot-granular prefix sums; the four 32-col groups of the PE
    are packed via tile_position.  The prefix table stays resident in SBUF
    as fp16 (rows duplicated to 128 wide for the transposing gather).
 2. Per-node edge means meanA[n] = T[hi_n] - T[lo_n], fetched feature-major
    with an SBUF-source transposing dma_gather, subtracted on DVE.
 3. The linear chain (node proj + edge proj + v-proj + attention scores) is
    folded host-side into W[192, 260]: vs = hT.T @ W[:128] + meanA.T.T @
    W[128:] -> [v(256) | scores(4)] per node, accumulated in PSUM.  All bias
    terms fold exactly: score constants cancel in the softmax; v constants
    pass through the pooling (sum p = 1) into the output bias.
 4. w = exp(scores) (segment-max skipped; scores are O(few), exp safe in
    fp32; softmax ratio unchanged).  Pooling = matmul with the per-chunk
    one-hot graph-membership matrix accumulated in PSUM -> [16, 260] of
    segment sums of [w*v | w].
 5. pooled = U/denom; out = pooled @ out_w.T + ob_eff (fp32).
"""
import sys

sys.path.insert(0, "/opt/trn_rl_repo")

import numpy as np

NUM_HEADS = 4
G_TOTAL = 128
CORES = 8
GL = G_TOTAL // CORES       # graphs per core
P = 128                     # partitions
SLOT = 4                    # edges per slot
SPC = 32                    # slots per chunk (128 edges)
CPG = 32                    # chunks per group (4096 edges)
GROUP_E = P * CPG           # 4096 edges per group
KB_LIST = [8, 8, 8, 7, 7, 6, 5, 3]   # node chunks per gather batch
S_BATCH = len(KB_LIST)
KB0 = [0]
for _kb in KB_LIST:
    KB0.append(KB0[-1] + _kb)
NC_NODES = KB0[-1] * P               # 6656 padded nodes per core
NKB = NC_NODES // P                  # 52 node chunks
LAG = 2                              # batches between gather issue and dense


# ----------------------------------------------------------------- host prep
def _pack_core(deg):
    """Pack node edge-runs (padded to 4-edge slots) into 128-edge chunks.

    Every chunk starts with one pad slot (so lo = start-1 stays in-chunk) and
    no run straddles a chunk; each 128-node block starts a fresh chunk.
    Returns (s0 slots [NL], r slots [NL], chunks per 128-node block [NB]).
    """
    NL = len(deg)
    r = (deg + SLOT - 1) // SLOT
    s0 = np.zeros(NL, np.int64)
    nblocks = (NL + P - 1) // P
    blk_chunks = np.zeros(nblocks, np.int64)
    cur = 0  # global slot cursor
    for kb in range(nblocks):
        if cur % SPC:
            cur = (cur // SPC + 1) * SPC
        start_chunk = cur // SPC
        blk_nodes = list(range(kb * P, min((kb + 1) * P, NL)))
        blk_nodes.sort(key=lambda n: -r[n])
        for n in blk_nodes:
            rn = r[n]
            if rn == 0:
                s0[n] = -1
                continue
            in_c = cur % SPC
            if in_c == 0:
                cur += 1
                in_c = 1
            if in_c + rn > SPC:
                cur = (cur // SPC + 1) * SPC + 1
            s0[n] = cur
            cur += rn
        blk_chunks[kb] = (cur + SPC - 1) // SPC - start_chunk
        cur = (cur + SPC - 1) // SPC * SPC
    return s0, r, blk_chunks


def _prep(h, edge_index, batch):
    """Shard + pack. Returns per-core dict of host arrays + shared config."""
    row = np.asarray(edge_index[0], np.int64)
    batch = np.asarray(batch, np.int64)
    # graph -> node range (batch is sorted)
    gstart = np.searchsorted(batch, np.arange(G_TOTAL + 1))
    order = np.argsort(row, kind="stable")
    row_s = row[order]

    cores = []
    for c in range(CORES):
        n0, n1 = int(gstart[GL * c]), int(gstart[GL * (c + 1)])
        NL = n1 - n0
        assert NL <= NC_NODES, (NL, NC_NODES)
        e0, e1 = np.searchsorted(row_s, [n0, n1])
        eord = order[e0:e1]
        lrow = row_s[e0:e1] - n0
        deg = np.bincount(lrow, minlength=NL)
        s0, r, blk_chunks = _pack_core(deg)
        need = np.zeros(S_BATCH, np.int64)
        for s in range(S_BATCH):
            ch = blk_chunks[KB0[s]:KB0[s + 1]].sum()
            need[s] = max(1, (ch + CPG - 1) // CPG)
        cores.append(dict(n0=n0, n1=n1, NL=NL, eord=eord, lrow=lrow, deg=deg,
                          s0=s0, r=r, blk_chunks=blk_chunks, need=need))
    B_S = [max(int(st["need"][s]) for st in cores) for s in range(S_BATCH)]
    G0 = [0]
    for b in B_S:
        G0.append(G0[-1] + b)
    NGROUPS = G0[-1]
    E_PAD = NGROUPS * GROUP_E

    for c, st in enumerate(cores):
        deg, s0, r, blk_chunks = st["deg"], st["s0"], st["r"], st["blk_chunks"]
        NL = st["NL"]
        # re-map block-local chunks to global chunks with batch alignment
        nb = len(blk_chunks)
        kb_batch = np.zeros(nb, np.int64)
        for s in range(S_BATCH):
            kb_batch[KB0[s]:KB0[s + 1]] = s
        blk_chunk0 = np.zeros(nb + 1, np.int64)
        cur_chunk = 0
        for kb in range(nb):
            s = int(kb_batch[kb])
            if kb == KB0[s]:
                cur_chunk = G0[s] * CPG
            blk_chunk0[kb] = cur_chunk
            cur_chunk += blk_chunks[kb]
            assert cur_chunk <= G0[s + 1] * CPG
        blk_chunk0[nb:] = cur_chunk
        # global slot of each node's run start (s0 was block-sequential)
        orig_start = np.zeros(nb, np.int64)
        acc = 0
        for kb in range(nb):
            orig_start[kb] = acc
            acc += blk_chunks[kb]
        shift = (blk_chunk0[:nb] - orig_start) * SPC  # slots to add per block
        node_blk = np.arange(NL) // P
        s0g = np.where(s0 >= 0, s0 + shift[node_blk], -1)

        # edge stream positions (logical), then swizzle to partition-major
        # DRAM layout so each SBUF partition reads one contiguous 2048B range
        # per group (cheap DMA descriptors).
        first_edge = np.concatenate([[0], np.cumsum(deg)])[:-1]
        epos_base = np.repeat(SLOT * s0g[deg > 0], deg[deg > 0])
        within = np.arange(len(st["lrow"])) - np.repeat(first_edge[deg > 0], deg[deg > 0])
        epos = epos_base + within
        assert epos.max(initial=-1) < E_PAD
        eg = epos // GROUP_E
        ec = (epos % GROUP_E) // P
        ep = epos % P
        epos = eg * GROUP_E + ep * CPG + ec

        # SBUF prefix-table rows for hi/lo slots (batch-relative).
        # Global slot gs: group g = gs//1024, local chunk lc = (gs%1024)//32,
        # sl = gs%32, m = lc//8, cp = lc%8.  Table row (within batch s) =
        # ((g - G0[s])*8 + cp)*128 + 32*m + sl; the gather resolves row ->
        # partition row%128, 256B rank row//128.
        g0_arr = np.asarray(G0[:-1])

        def slot_to_row(sl_g, batch_of_node):
            g = sl_g // (CPG * SPC)
            ws = sl_g % (CPG * SPC)
            lc = ws // SPC
            sl = ws % SPC
            m, cp = lc // 8, lc % 8
            grel = g - g0_arr[batch_of_node]
            return ((grel * 8 + cp) * P + 32 * m + sl)

        nbatch = np.searchsorted(np.asarray(KB0[1:]), np.arange(NL) // P,
                                 side="right")
        hi = np.where(s0g >= 0, s0g + r - 1, 0)
        lo = np.where(s0g >= 0, s0g - 1, 0)
        hi_row = np.where(s0g >= 0, slot_to_row(hi, nbatch), 0).astype(np.int64)
        lo_row = np.where(s0g >= 0, slot_to_row(lo, nbatch), 0).astype(np.int64)
        hi_row = np.pad(hi_row, (0, NC_NODES - NL))
        lo_row = np.pad(lo_row, (0, NC_NODES - NL))
        for s in range(S_BATCH):
            nsl = slice(KB0[s] * P, KB0[s + 1] * P)
            assert hi_row[nsl].max() < B_S[s] * 1024
            assert lo_row[nsl].max() < B_S[s] * 1024

        st.update(epos=epos, hi_row=hi_row, lo_row=lo_row)
    cfg = dict(B_S=B_S, G0=G0, NGROUPS=NGROUPS, E_PAD=E_PAD)
    return cores, cfg


def _wrap_idx(a, npart_rep=8):
    """[M] -> [128, M//16] int16, F-wrapped 16-row block replicated 8x."""
    m = a.reshape(-1, 16).T.astype(np.int16)          # [16, M/16]
    return np.tile(m, (npart_rep, 1))


def _fold_weights(node_w, node_b, edge_w, edge_b, query, in_w, in_b, out_w, out_b):
    D = query.shape[-1]
    dh = D // NUM_HEADS
    wq, wk, wv = in_w[:D], in_w[D:2 * D], in_w[2 * D:]
    bq, bk, bv = in_b[:D], in_b[D:2 * D], in_b[2 * D:]
    q = (query[0] @ wq.T + bq).reshape(NUM_HEADS, dh)
    s_w = np.einsum("hj,hjd->dh", q, wk.reshape(NUM_HEADS, dh, D)) / np.sqrt(dh)
    # x = [h(128) | meanA(64)] -> [v(256) | scores(4)]; bias rows fold exactly:
    # score constants cancel in the softmax, v constants pass through the
    # pooling (sum p = 1) into ob_eff.  hase*edge_b terms need a device-side
    # per-node flag only when edge_b != 0 (wb3 path).
    A1 = np.concatenate([node_w.T, edge_w.T], axis=0)   # [192, 256]
    M2 = np.concatenate([wv.T, s_w], axis=1)            # [256, 260]
    Wbig = (A1 @ M2).astype(np.float32)                 # [192, 260]
    vconst = node_b @ wv.T + bv                         # [256]
    ob_eff = (out_b + vconst @ out_w.T).astype(np.float32)
    wb3 = (edge_b @ M2).astype(np.float32)              # [260] hase row
    return Wbig, ob_eff, wb3


# ------------------------------------------------------- numpy device model
def _numpy_device_model(cores, cfg, h, edge_attr, batch, Wbig, wb3, use_wb3,
                        ob_eff, out_w, out_b):
    """Bit-approximate emulation of the device program — used to validate
    packing/indexing host-side."""
    import ml_dtypes
    bf = lambda x: x.astype(ml_dtypes.bfloat16).astype(np.float32)
    f16 = lambda x: x.astype(np.float16).astype(np.float32)
    f8 = lambda x: x.astype(ml_dtypes.float8_e4m3fn).astype(np.float32)
    B_S, G0, NGROUPS, E_PAD = cfg["B_S"], cfg["G0"], cfg["NGROUPS"], cfg["E_PAD"]
    outs = []
    for c, st in enumerate(cores):
        NL = st["NL"]
        deg = np.pad(st["deg"], (0, NC_NODES - NL)).astype(np.float32)
        inv = 1.0 / np.maximum(deg, 1.0)
        stream = np.zeros((E_PAD, 64), np.float32)
        stream[st["epos"]] = edge_attr[st["eord"]] * inv[st["lrow"]][:, None]
        streamb = f8(stream)
        # fp16 prefix table, one row per (group, lc, sl)
        tab = np.zeros((NGROUPS * 1024, 64), np.float32)
        for g in range(NGROUPS):
            for lc in range(CPG):
                cdat = streamb[g * GROUP_E + np.arange(P) * CPG + lc]
                pre = np.add.reduceat(cdat, np.arange(0, P, SLOT), 0).cumsum(0)
                m, cp = lc // 8, lc % 8
                sl = np.arange(SPC)
                tab[(g * 8 + cp) * P + 32 * m + sl] = pre
        tab = f16(tab)
        nbatch = np.searchsorted(np.asarray(KB0[1:]), np.arange(NC_NODES) // P,
                                 side="right")
        base = np.asarray(G0)[nbatch] * 1024
        phi = tab[base + st["hi_row"]]
        plo = tab[base + st["lo_row"]]
        meanA = f16(phi - plo)
        hpad = np.zeros((NC_NODES, 128), np.float32)
        hpad[:NL] = h[st["n0"]:st["n1"]]
        vs = bf(hpad) @ bf(Wbig[:128]) + meanA @ f16(Wbig[128:])
        if use_wb3:
            hase = (deg > 0).astype(np.float32)
            vs = vs + hase[:, None] * f16(wb3)[None, :]
        v, sc = vs[:, :256], vs[:, 256:]
        w = np.exp(sc)
        bl = np.full(NC_NODES, -1, np.int64)
        bl[:NL] = batch[st["n0"]:st["n1"]] - GL * c
        onehot = (bl[:, None] == np.arange(GL)[None, :]).astype(np.float32)
        wv4 = np.concatenate([bf(w[:, :, None] * v.reshape(-1, 4, 64)).reshape(-1, 256),
                              bf(w)], 1)
        U = bf(onehot).T @ wv4
        den = np.maximum(U[:, 256:], 1e-30)
        pooled = U[:, :256].reshape(GL, 4, 64) / den[:, :, None]
        o = pooled.reshape(GL, 256) @ out_w.T + ob_eff
        outs.append(o)
    return np.concatenate(outs).reshape(G_TOTAL, 1, 256)


# ------------------------------------------------------------- bass program
def _build_program(cfg, use_wb3, debug=False):
    import concourse.bacc as bacc
    import concourse.mybir as mybir
    import concourse.tile as tile

    F32 = mybir.dt.float32
    BF16 = mybir.dt.bfloat16
    F16 = mybir.dt.float16
    FP8 = mybir.dt.float8e4
    I16 = mybir.dt.int16
    AF = mybir.ActivationFunctionType
    B_S, G0, NGROUPS, E_PAD = cfg["B_S"], cfg["G0"], cfg["NGROUPS"], cfg["E_PAD"]
    IDX_COLS = sum(2 * KB_LIST[s] * 8 for s in range(S_BATCH))

    nc = bacc.Bacc("TRN2", num_devices=CORES, num_swdge_queues=4)
    es_d = nc.dram_tensor("es", [E_PAD, 64], FP8, kind="ExternalInput")
    h_d = nc.dram_tensor("h", [P, NKB, 128], BF16, kind="ExternalInput")
    mem_d = nc.dram_tensor("mem", [P, NKB, GL], BF16, kind="ExternalInput")
    idx_d = nc.dram_tensor("idx", [P, IDX_COLS], I16, kind="ExternalInput")
    tri_d = nc.dram_tensor("tri", [P, SPC], FP8, kind="ExternalInput")
    idtf_d = nc.dram_tensor("idtf", [GL, GL], F32, kind="ExternalInput")
    wb1_d = nc.dram_tensor("wb1", [128, 260], BF16, kind="ExternalInput")
    wb2_d = nc.dram_tensor("wb2", [64, 260], F16, kind="ExternalInput")
    owt_d = nc.dram_tensor("owt", [256, 256], F32, kind="ExternalInput")
    ob_d = nc.dram_tensor("ob", [GL, 256], F32, kind="ExternalInput")
    if use_wb3:
        wb3_d = nc.dram_tensor("wb3", [1, 260], F16, kind="ExternalInput")
        cst_d = nc.dram_tensor("cst", [1, NC_NODES], F16, kind="ExternalInput")
    y_d = nc.dram_tensor("y", [GL, 256], F32, kind="ExternalOutput")
    if debug:
        tabout_d = nc.dram_tensor("tabout", [P, NGROUPS * 8, 2, 64], F16,
                                  kind="ExternalOutput")
        amout_d = nc.dram_tensor("amout", [S_BATCH, 64, 8 * P], F16,
                                 kind="ExternalOutput")
        xgout_d = nc.dram_tensor("xgout", [2, P, 2 * 8 * P], F16,
                                 kind="ExternalOutput")
        xgout2_d = nc.dram_tensor("xgout2", [2, P, 2 * 8 * P], F16,
                                  kind="ExternalOutput")

    with tile.TileContext(nc) as tc:
        with tc.tile_pool(name="const", bufs=1) as cp, \
             tc.tile_pool(name="sb", bufs=3) as sb, \
             tc.tile_pool(name="big", bufs=1) as bigp, \
             tc.tile_pool(name="ps", bufs=2, space="PSUM") as ps, \
             tc.tile_pool(name="pacc", bufs=1, space="PSUM") as pacc:

            trib = cp.tile([P, SPC], FP8, name="trib")
            nc.sync.dma_start(out=trib[:], in_=tri_d.ap()[:, :])
            idtf = cp.tile([GL, GL], F32, name="idtf")
            nc.sync.dma_start(out=idtf[:], in_=idtf_d.ap()[:, :])
            wb1 = cp.tile([128, 260], BF16, name="wb1")
            nc.sync.dma_start(out=wb1[:], in_=wb1_d.ap()[:, :])
            wb2 = cp.tile([64, 260], F16, name="wb2")
            nc.sync.dma_start(out=wb2[:], in_=wb2_d.ap()[:, :])
            owt = cp.tile([P, 2, 256], F32, name="owt")
            nc.sync.dma_start(out=owt[:], in_=owt_d.ap()[:, :].rearrange("(i p) f -> p i f", p=P))
            obt = cp.tile([GL, 256], F32, name="obt")
            nc.sync.dma_start(out=obt[:], in_=ob_d.ap()[:, :])
            idxt = cp.tile([P, IDX_COLS], I16, name="idxt")
            nc.sync.dma_start(out=idxt[:], in_=idx_d.ap()[:, :])
            if use_wb3:
                wb3 = cp.tile([1, 260], F16, name="wb3")
                nc.sync.dma_start(out=wb3[:], in_=wb3_d.ap()[:, :])
                cst = cp.tile([1, NC_NODES], F16, name="cst")
                nc.sync.dma_start(out=cst[:], in_=cst_d.ap()[:, :])

            # h pre-transposed: partition = feature, free = (chunk, node);
            # big loads go on the scalar HWDGE queue, off the edge stream.
            hsb = bigp.tile([P, NKB, 128], BF16, name="hsb")
            nc.scalar.dma_start(out=hsb[:], in_=h_d.ap()[:, :, :])
            memall = bigp.tile([P, NKB, GL], BF16, name="memall")
            nc.scalar.dma_start(out=memall[:], in_=mem_d.ap()[:, :, :])

            # fp16 prefix table, SBUF-resident, rows duplicated to 256B for
            # the transposing gather: row ridx -> partition ridx%128, rank
            # ridx//128 (= [group*8 + chunk%8] majors).
            table = bigp.tile([P, NGROUPS * 8, 2, 64], F16, name="table")

            pool_ps = pacc.tile([GL, 260], F32, name="pool_ps")

            xg_tiles = {}
            am_tiles = {}
            idx_off = [0]
            for s in range(S_BATCH):
                idx_off.append(idx_off[-1] + 2 * KB_LIST[s] * 8)

            def emit_gather(s):
                KBB = KB_LIST[s]
                nidx = 2 * KBB * P
                xg = sb.tile([P, 1, 2 * 8 * P], F16, name="xg", tag="xg", bufs=3)
                nc.gpsimd.dma_gather(
                    out_ap=xg[:, :, :nidx],
                    in_ap=table[:, G0[s] * 8:G0[s + 1] * 8, :, :],
                    idxs_ap=idxt[:, idx_off[s]:idx_off[s + 1]],
                    num_idxs=nidx, num_idxs_reg=nidx, elem_size=128,
                    transpose=True, single_packet=False, queue_num=0,
                    sbuf_tokens_per_rank=128,
                    sbuf_free_dim_per_rank=256)
                am = sb.tile([64, 8 * P], F16, name="am", tag="am", bufs=LAG + 1)
                nc.vector.tensor_sub(out=am[:, :KBB * P],
                                     in0=xg[0:64, 0, :KBB * P],
                                     in1=xg[0:64, 0, KBB * P:nidx])
                if debug:
                    nc.sync.dma_start(out=amout_d.ap()[s, :, :KBB * P],
                                      in_=am[:, :KBB * P])
                    if s < 2:
                        nc.sync.dma_start(out=xgout_d.ap()[s, :, :nidx],
                                          in_=xg[:, 0, :nidx])
                xg_tiles[s] = xg
                am_tiles[s] = am

            def emit_dense(s):
                k0, KBB = KB0[s], KB_LIST[s]
                am = am_tiles[s]
                for k in range(k0, k0 + KBB):
                    j = k - k0
                    vs = ps.tile([P, 260], F32, name="vs", tag="vs", bufs=3)
                    nc.tensor.matmul(out=vs[:], lhsT=hsb[:, k, :], rhs=wb1[:],
                                     start=True, stop=False)
                    nc.tensor.matmul(out=vs[:], lhsT=am[:, j * P:(j + 1) * P],
                                     rhs=wb2[:], start=False, stop=not use_wb3)
                    if use_wb3:
                        nc.tensor.matmul(out=vs[:], lhsT=cst[:, k * P:(k + 1) * P],
                                         rhs=wb3[:], start=False, stop=True)
                    wsb = sb.tile([P, 4], F32, name="wsb", tag="wsb", bufs=4)
                    nc.scalar.activation(out=wsb[:], in_=vs[:, 256:260], func=AF.Exp)
                    pr = sb.tile([P, 260], BF16, name="pr", tag="pr", bufs=4)
                    nc.vector.tensor_tensor(
                        out=pr[:, :128].rearrange("p (h f) -> p h f", h=2),
                        in0=vs[:, :128].rearrange("p (h f) -> p h f", h=2),
                        in1=wsb[:, 0:2].broadcast_to([P, 2, 64]),
                        op=mybir.AluOpType.mult)
                    nc.scalar.activation(out=pr[:, 128:192], in_=vs[:, 128:192],
                                         func=AF.Copy, scale=wsb[:, 2:3])
                    nc.scalar.activation(out=pr[:, 192:256], in_=vs[:, 192:256],
                                         func=AF.Copy, scale=wsb[:, 3:4])
                    nc.scalar.copy(out=pr[:, 256:260], in_=wsb[:])
                    nc.tensor.matmul(out=pool_ps[:], lhsT=memall[:, k, :], rhs=pr[:],
                                     start=(k == 0), stop=(k == NKB - 1))

            import bisect
            for g in range(NGROUPS):
                et = sb.tile([P, CPG, 64], FP8, name="et", tag="et", bufs=6)
                nc.sync.dma_start(
                    out=et[:],
                    in_=es_d.ap()[g * GROUP_E:(g + 1) * GROUP_E, :]
                        .rearrange("(p c) f -> p c f", p=P))
                pp = ps.tile([P, 512], F32, name="pp", tag="pp", bufs=2)
                for m in range(4):
                    nc.tensor.matmul(
                        out=pp[32 * m:32 * m + 32, :],
                        lhsT=trib[:],
                        rhs=et[:, 8 * m:8 * m + 8, :].rearrange("p c f -> p (c f)"),
                        start=True, stop=True,
                        tile_position=(0, 32 * m))
                eng = nc.vector if g % 2 == 0 else nc.scalar
                src = pp[:].rearrange("p (c f) -> p c f", c=8) \
                           .unsqueeze(2).broadcast_to([P, 8, 2, 64])
                if g % 2 == 0:
                    eng.tensor_copy(out=table[:, g * 8:(g + 1) * 8, :, :], in_=src)
                else:
                    eng.copy(out=table[:, g * 8:(g + 1) * 8, :, :], in_=src)
                s = bisect.bisect_right(G0, g) - 1
                if g == G0[s + 1] - 1:
                    emit_gather(s)
                    if s >= LAG:
                        emit_dense(s - LAG)

            for s in range(max(0, S_BATCH - LAG), S_BATCH):
                emit_dense(s)
            if debug:
                nc.sync.dma_start(out=tabout_d.ap()[:, :, :, :], in_=table[:])
                # late re-gather of batches 0/1: distinguishes timing race
                # from deterministic addressing bugs
                for s in range(2):
                    KBB = KB_LIST[s]
                    nidx = 2 * KBB * P
                    xg2 = sb.tile([P, 1, 2 * 8 * P], F16, name="xg2", tag="xg",
                                  bufs=3)
                    nc.gpsimd.dma_gather(
                        out_ap=xg2[:, :, :nidx],
                        in_ap=table[:, G0[s] * 8:G0[s + 1] * 8, :, :],
                        idxs_ap=idxt[:, idx_off[s]:idx_off[s + 1]],
                        num_idxs=nidx, num_idxs_reg=nidx, elem_size=128,
                        transpose=True, single_packet=False, queue_num=0,
                        sbuf_tokens_per_rank=128,
                        sbuf_free_dim_per_rank=256)
                    nc.sync.dma_start(out=xgout2_d.ap()[s, :, :nidx],
                                      in_=xg2[:, 0, :nidx])

            # ---- final: normalize + output projection
            den = sb.tile([GL, 4], F32, name="den")
            nc.vector.tensor_scalar_max(out=den[:], in0=pool_ps[:, 256:260],
                                        scalar1=1e-30)
            rden = sb.tile([GL, 4], F32, name="rden")
            nc.vector.reciprocal(out=rden[:], in_=den[:])
            pn = sb.tile([GL, 256], F32, name="pn")
            for hh in range(NUM_HEADS):
                nc.vector.tensor_scalar_mul(out=pn[:, 64 * hh:64 * hh + 64],
                                            in0=pool_ps[:, 64 * hh:64 * hh + 64],
                                            scalar1=rden[:, hh:hh + 1])
            pnT = sb.tile([P, 2, GL], F32, name="pnT")
            for i in range(2):
                ptp = ps.tile([P, GL], F32, name="ptp", tag="vs", bufs=3)
                nc.tensor.transpose(out=ptp[:], in_=pn[:, i * P:(i + 1) * P],
                                    identity=idtf[:])
                nc.vector.tensor_copy(out=pnT[:, i, :], in_=ptp[:])
            ops_t = ps.tile([GL, 256], F32, name="ops_t", tag="vs", bufs=3)
            for i in range(2):
                nc.tensor.matmul(out=ops_t[:], lhsT=pnT[:, i, :], rhs=owt[:, i, :],
                                 start=(i == 0), stop=(i == 1))
            osb = sb.tile([GL, 256], F32, name="osb")
            nc.vector.tensor_add(out=osb[:], in0=ops_t[:], in1=obt[:])
            nc.sync.dma_start(out=y_d.ap()[:, :], in_=osb[:])

    nc.finalize()
    return nc


_CACHE = {}


def _get_program(cfg, use_wb3):
    key = (tuple(cfg["B_S"]), use_wb3)
    if key not in _CACHE:
        _CACHE[key] = _build_program(cfg, use_wb3)
    return _CACHE[key]


def kernel(h, edge_index, edge_attr, batch, num_graphs,
           node_w, node_b, edge_w, edge_b, query, in_w, in_b, out_w, out_b,
           _trace=False, _numpy_only=False):
    import ml_dtypes
    h = np.asarray(h, np.float32)
    edge_attr = np.asarray(edge_attr, np.float32)
    batch_np = np.asarray(batch, np.int64)
    assert int(num_graphs) == G_TOTAL

    cores, cfg = _prep(h, edge_index, batch_np)
    Wbig, ob_eff, wb3 = _fold_weights(
        np.asarray(node_w, np.float32), np.asarray(node_b, np.float32),
        np.asarray(edge_w, np.float32), np.asarray(edge_b, np.float32),
        np.asarray(query, np.float32), np.asarray(in_w, np.float32),
        np.asarray(in_b, np.float32), np.asarray(out_w, np.float32),
        np.asarray(out_b, np.float32))
    use_wb3 = bool(np.any(np.asarray(edge_b, np.float32) != 0))
    if _numpy_only:
        return _numpy_device_model(cores, cfg, h, edge_attr, batch_np, Wbig,
                                   wb3, use_wb3, ob_eff,
                                   np.asarray(out_w, np.float32),
                                   np.asarray(out_b, np.float32))

    bf16 = ml_dtypes.bfloat16
    fp8 = ml_dtypes.float8_e4m3fn
    tri = (np.arange(P)[:, None] // SLOT <= np.arange(SPC)[None, :]).astype(np.float32)
    shared = dict(
        tri=tri.astype(fp8),
        idtf=np.eye(GL, dtype=np.float32),
        wb1=Wbig[:128].astype(bf16), wb2=Wbig[128:].astype(np.float16),
        owt=np.ascontiguousarray(np.asarray(out_w, np.float32).T),
        ob=np.tile(ob_eff[None, :], (GL, 1)).astype(np.float32),
    )
    if use_wb3:
        shared["wb3"] = wb3[None, :].astype(np.float16)
    in_maps = []
    for c, st in enumerate(cores):
        NL = st["NL"]
        deg = np.pad(st["deg"], (0, NC_NODES - NL)).astype(np.float32)
        inv = 1.0 / np.maximum(deg, 1.0)
        stream = np.zeros((cfg["E_PAD"], 64), np.float32)
        stream[st["epos"]] = edge_attr[st["eord"]] * inv[st["lrow"]][:, None]
        hpad = np.zeros((NC_NODES, 128), np.float32)
        hpad[:NL] = h[st["n0"]:st["n1"]]
        h3 = np.ascontiguousarray(hpad.reshape(-1, P, 128).transpose(2, 0, 1))
        bl = np.full(NC_NODES, -1, np.int64)
        bl[:NL] = batch_np[st["n0"]:st["n1"]] - GL * c
        mem = (bl.reshape(NKB, P)[:, :, None] == np.arange(GL)[None, None, :])
        mem3 = np.ascontiguousarray(mem.transpose(1, 0, 2)).astype(bf16)
        idx_parts = []
        for s in range(S_BATCH):
            nsl = slice(KB0[s] * P, KB0[s + 1] * P)
            idx_parts.append(st["hi_row"][nsl])
            idx_parts.append(st["lo_row"][nsl])
        idx = _wrap_idx(np.concatenate(idx_parts))
        im = dict(es=stream.astype(fp8), h=h3.astype(bf16), mem=mem3,
                  idx=idx, **shared)
        if use_wb3:
            im["cst"] = (deg > 0).astype(np.float16)[None, :]
        in_maps.append(im)

    from concourse.bass_utils import run_bass_kernel_spmd
    nc = _get_program(cfg, use_wb3)
    res = run_bass_kernel_spmd(nc, in_maps, core_ids=list(range(CORES)),
                               trace=_trace)
    out = np.concatenate([np.asarray(res.results[c]["y"], np.float32)
                          for c in range(CORES)])
    kernel._last_result = res
    return out.reshape(G_TOTAL, 1, 256)


# revision 31
# speedup vs baseline: 1.3095x; 1.3095x over previous
"""AttentionPooling GNN kernel for 8 Trainium2 NeuronCores.

Strategy
--------
Graph-parallel sharding: 128 graphs -> 16 graphs per core; each core gets its
graphs' nodes and (re-grouped) edges.  Host does index preprocessing only
(edge permutation by source node, degree counts, weight folding); all FLOPs
on the edge/node payloads run on device.

Device algorithm (per core):
 1. Edges arrive fp8, pre-scaled by 1/deg(src), sorted by source node and
    padded so each node's run occupies whole 4-edge slots inside a single
    128-edge chunk.  A shared triangular stationary (TriU4, fp8) turns chunk
    matmuls into slot-granular prefix sums; the four 32-col groups of the PE
    are packed via tile_position.  The prefix table stays resident in SBUF
    as fp16 (rows duplicated to 128 wide for the transposing gather).
 2. Per-node edge means meanA[n] = T[hi_n] - T[lo_n], fetched feature-major
    with an SBUF-source transposing dma_gather, subtracted on DVE.
 3. The linear chain (node proj + edge proj + v-proj + attention scores) is
    folded host-side into W[192, 260]: vs = hT.T @ W[:128] + meanA.T.T @
    W[128:] -> [v(256) | scores(4)] per node, accumulated in PSUM.  All bias
    terms fold exactly: score constants cancel in the softmax; v constants
    pass through the pooling (sum p = 1) into the output bias.
 4. w = exp(scores) (segment-max skipped; scores are O(few), exp safe in
    fp32; softmax ratio unchanged).  Pooling = matmul with the per-chunk
    one-hot graph-membership matrix accumulated in PSUM -> [16, 260] of
    segment sums of [w*v | w].
 5. pooled = U/denom; out = pooled @ out_w.T + ob_eff (fp32).
"""
import sys

sys.path.insert(0, "/opt/trn_rl_repo")

import numpy as np

NUM_HEADS = 4
G_TOTAL = 128
CORES = 8
GL = G_TOTAL // CORES       # graphs per core
P = 128                     # partitions
SLOT = 4                    # edges per slot
SPC = 32                    # slots per chunk (128 edges)
CPG = 32                    # chunks per group (4096 edges)
GROUP_E = P * CPG           # 4096 edges per group
KB_LIST = [8, 8, 8, 7, 7, 6, 5, 3]   # node chunks per gather batch
S_BATCH = len(KB_LIST)
KB0 = [0]
for _kb in KB_LIST:
    KB0.append(KB0[-1] + _kb)
NC_NODES = KB0[-1] * P               # 6656 padded nodes per core
NKB = NC_NODES // P                  # 52 node chunks
LAG = 2                              # batches between gather issue and dense


# ----------------------------------------------------------------- host prep
def _pack_core(deg):
    """Pack node edge-runs (padded to 4-edge slots) into 128-edge chunks.

    Every chunk starts with one pad slot (so lo = start-1 stays in-chunk) and
    no run straddles a chunk; each 128-node block starts a fresh chunk.
    Returns (s0 slots [NL], r slots [NL], chunks per 128-node block [NB]).
    """
    NL = len(deg)
    r = (deg + SLOT - 1) // SLOT
    s0 = np.zeros(NL, np.int64)
    nblocks = (NL + P - 1) // P
    blk_chunks = np.zeros(nblocks, np.int64)
    cur = 0  # global slot cursor
    for kb in range(nblocks):
        if cur % SPC:
            cur = (cur // SPC + 1) * SPC
        start_chunk = cur // SPC
        blk_nodes = list(range(kb * P, min((kb + 1) * P, NL)))
        blk_nodes.sort(key=lambda n: -r[n])
        for n in blk_nodes:
            rn = r[n]
            if rn == 0:
                s0[n] = -1
                continue
            in_c = cur % SPC
            if in_c == 0:
                cur += 1
                in_c = 1
            if in_c + rn > SPC:
                cur = (cur // SPC + 1) * SPC + 1
            s0[n] = cur
            cur += rn
        blk_chunks[kb] = (cur + SPC - 1) // SPC - start_chunk
        cur = (cur + SPC - 1) // SPC * SPC
    return s0, r, blk_chunks


def _prep(h, edge_index, batch):
    """Shard + pack. Returns per-core dict of host arrays + shared config."""
    row = np.asarray(edge_index[0], np.int64)
    batch = np.asarray(batch, np.int64)
    # graph -> node range (batch is sorted)
    gstart = np.searchsorted(batch, np.arange(G_TOTAL + 1))
    order = np.argsort(row, kind="stable")
    row_s = row[order]

    cores = []
    for c in range(CORES):
        n0, n1 = int(gstart[GL * c]), int(gstart[GL * (c + 1)])
        NL = n1 - n0
        assert NL <= NC_NODES, (NL, NC_NODES)
        e0, e1 = np.searchsorted(row_s, [n0, n1])
        eord = order[e0:e1]
        lrow = row_s[e0:e1] - n0
        deg = np.bincount(lrow, minlength=NL)
        s0, r, blk_chunks = _pack_core(deg)
        need = np.zeros(S_BATCH, np.int64)
        for s in range(S_BATCH):
            ch = blk_chunks[KB0[s]:KB0[s + 1]].sum()
            need[s] = max(1, (ch + CPG - 1) // CPG)
        cores.append(dict(n0=n0, n1=n1, NL=NL, eord=eord, lrow=lrow, deg=deg,
                          s0=s0, r=r, blk_chunks=blk_chunks, need=need))
    B_S = [max(int(st["need"][s]) for st in cores) for s in range(S_BATCH)]
    G0 = [0]
    for b in B_S:
        G0.append(G0[-1] + b)
    NGROUPS = G0[-1]
    E_PAD = NGROUPS * GROUP_E

    for c, st in enumerate(cores):
        deg, s0, r, blk_chunks = st["deg"], st["s0"], st["r"], st["blk_chunks"]
        NL = st["NL"]
        # re-map block-local chunks to global chunks with batch alignment
        nb = len(blk_chunks)
        kb_batch = np.zeros(nb, np.int64)
        for s in range(S_BATCH):
            kb_batch[KB0[s]:KB0[s + 1]] = s
        blk_chunk0 = np.zeros(nb + 1, np.int64)
        cur_chunk = 0
        for kb in range(nb):
            s = int(kb_batch[kb])
            if kb == KB0[s]:
                cur_chunk = G0[s] * CPG
            blk_chunk0[kb] = cur_chunk
            cur_chunk += blk_chunks[kb]
            assert cur_chunk <= G0[s + 1] * CPG
        blk_chunk0[nb:] = cur_chunk
        # global slot of each node's run start (s0 was block-sequential)
        orig_start = np.zeros(nb, np.int64)
        acc = 0
        for kb in range(nb):
            orig_start[kb] = acc
            acc += blk_chunks[kb]
        shift = (blk_chunk0[:nb] - orig_start) * SPC  # slots to add per block
        node_blk = np.arange(NL) // P
        s0g = np.where(s0 >= 0, s0 + shift[node_blk], -1)

        # edge stream positions (logical), then swizzle to partition-major
        # DRAM layout so each SBUF partition reads one contiguous 2048B range
        # per group (cheap DMA descriptors).
        first_edge = np.concatenate([[0], np.cumsum(deg)])[:-1]
        epos_base = np.repeat(SLOT * s0g[deg > 0], deg[deg > 0])
        within = np.arange(len(st["lrow"])) - np.repeat(first_edge[deg > 0], deg[deg > 0])
        epos = epos_base + within
        assert epos.max(initial=-1) < E_PAD
        eg = epos // GROUP_E
        ec = (epos % GROUP_E) // P
        ep = epos % P
        epos = eg * GROUP_E + ep * CPG + ec

        # SBUF prefix-table rows for hi/lo slots (batch-relative).
        # Global slot gs: group g = gs//1024, local chunk lc = (gs%1024)//32,
        # sl = gs%32, m = lc//8, cp = lc%8.  Table row (within batch s) =
        # ((g - G0[s])*8 + cp)*128 + 32*m + sl; the gather resolves row ->
        # partition row%128, 256B rank row//128.
        g0_arr = np.asarray(G0[:-1])

        def slot_to_row(sl_g, batch_of_node):
            g = sl_g // (CPG * SPC)
            ws = sl_g % (CPG * SPC)
            lc = ws // SPC
            sl = ws % SPC
            m, cp = lc // 8, lc % 8
            grel = g - g0_arr[batch_of_node]
            return ((grel * 8 + cp) * P + 32 * m + sl)

        # Permute nodes within each 128-block into placement order (the
        # packer's descending-r order).  Pooling/memall are order-invariant,
        # so the permutation is free; it makes lo_row(j) == hi_row(j-1) for
        # consecutive same-chunk nodes, so only hi rows need gathering --
        # plo becomes a shifted view with a chunk-start mask.
        perm = np.arange(NC_NODES)
        for kb in range((NL + P - 1) // P):
            blk = list(range(kb * P, min((kb + 1) * P, NL)))
            blk.sort(key=lambda n: -r[n])
            perm[kb * P:kb * P + len(blk)] = blk
        real = perm < NL
        safe = np.minimum(perm, max(NL - 1, 0))
        s0p = np.where(real, s0g[safe], -1)
        rp = np.where(real, r[safe], 0)

        nbatch = np.searchsorted(np.asarray(KB0[1:]), np.arange(NC_NODES) // P,
                                 side="right")
        hi = np.where(s0p >= 0, s0p + rp - 1, 0)
        hi_row = np.where(s0p >= 0, slot_to_row(hi, nbatch), 0).astype(np.int64)
        chunk_id = np.where(s0p >= 0, s0p // SPC, -1)
        mask = np.zeros(NC_NODES, np.float32)
        j = np.arange(1, NC_NODES)
        same = (chunk_id[j] >= 0) & (chunk_id[j] == chunk_id[j - 1]) & (j % P != 0)
        mask[j] = same.astype(np.float32)
        # invariants: same-chunk consecutive placement is gapless; chunk-first
        # runs start at slot 1 (so their prefix-before is the zero pad slot)
        assert np.all(s0p[j][same] == s0p[j - 1][same] + rp[j - 1][same])
        first = (chunk_id >= 0) & (mask == 0)
        assert np.all(s0p[first] % SPC == 1)
        for s in range(S_BATCH):
            nsl = slice(KB0[s] * P, KB0[s + 1] * P)
            assert hi_row[nsl].max() < B_S[s] * 1024

        st.update(epos=epos, hi_row=hi_row, perm=perm, mask=mask)
    cfg = dict(B_S=B_S, G0=G0, NGROUPS=NGROUPS, E_PAD=E_PAD)
    return cores, cfg


def _wrap_idx(a, npart_rep=8):
    """[M] -> [128, M//16] int16, F-wrapped 16-row block replicated 8x."""
    m = a.reshape(-1, 16).T.astype(np.int16)          # [16, M/16]
    return np.tile(m, (npart_rep, 1))


def _fold_weights(node_w, node_b, edge_w, edge_b, query, in_w, in_b, out_w, out_b):
    D = query.shape[-1]
    dh = D // NUM_HEADS
    wq, wk, wv = in_w[:D], in_w[D:2 * D], in_w[2 * D:]
    bq, bk, bv = in_b[:D], in_b[D:2 * D], in_b[2 * D:]
    q = (query[0] @ wq.T + bq).reshape(NUM_HEADS, dh)
    s_w = np.einsum("hj,hjd->dh", q, wk.reshape(NUM_HEADS, dh, D)) / np.sqrt(dh)
    # x = [h(128) | meanA(64)] -> [v(256) | scores(4)]; bias rows fold exactly:
    # score constants cancel in the softmax, v constants pass through the
    # pooling (sum p = 1) into ob_eff.  hase*edge_b terms need a device-side
    # per-node flag only when edge_b != 0 (wb3 path).
    A1 = np.concatenate([node_w.T, edge_w.T], axis=0)   # [192, 256]
    M2 = np.concatenate([wv.T, s_w], axis=1)            # [256, 260]
    Wbig = (A1 @ M2).astype(np.float32)                 # [192, 260]
    vconst = node_b @ wv.T + bv                         # [256]
    ob_eff = (out_b + vconst @ out_w.T).astype(np.float32)
    wb3 = (edge_b @ M2).astype(np.float32)              # [260] hase row
    return Wbig, ob_eff, wb3


# ------------------------------------------------------- numpy device model
def _numpy_device_model(cores, cfg, h, edge_attr, batch, Wbig, wb3, use_wb3,
                        ob_eff, out_w, out_b):
    """Bit-approximate emulation of the device program — used to validate
    packing/indexing host-side."""
    import ml_dtypes
    bf = lambda x: x.astype(ml_dtypes.bfloat16).astype(np.float32)
    f16 = lambda x: x.astype(np.float16).astype(np.float32)
    f8 = lambda x: x.astype(ml_dtypes.float8_e4m3fn).astype(np.float32)
    B_S, G0, NGROUPS, E_PAD = cfg["B_S"], cfg["G0"], cfg["NGROUPS"], cfg["E_PAD"]
    outs = []
    for c, st in enumerate(cores):
        NL = st["NL"]
        deg = np.pad(st["deg"], (0, NC_NODES - NL)).astype(np.float32)
        inv = 1.0 / np.maximum(deg, 1.0)
        stream = np.zeros((E_PAD, 64), np.float32)
        stream[st["epos"]] = edge_attr[st["eord"]] * inv[st["lrow"]][:, None]
        streamb = f8(stream)
        # fp16 prefix table, one row per (group, lc, sl)
        tab = np.zeros((NGROUPS * 1024, 64), np.float32)
        for g in range(NGROUPS):
            for lc in range(CPG):
                cdat = streamb[g * GROUP_E + np.arange(P) * CPG + lc]
                pre = np.add.reduceat(cdat, np.arange(0, P, SLOT), 0).cumsum(0)
                m, cp = lc // 8, lc % 8
                sl = np.arange(SPC)
                tab[(g * 8 + cp) * P + 32 * m + sl] = pre
        tab = f16(tab)
        nbatch = np.searchsorted(np.asarray(KB0[1:]), np.arange(NC_NODES) // P,
                                 side="right")
        base = np.asarray(G0)[nbatch] * 1024
        phi = tab[base + st["hi_row"]]                  # [NC_NODES, 64] placed
        mask = st["mask"]
        plo = np.zeros_like(phi)
        plo[1:] = phi[:-1] * mask[1:, None]
        meanA = f16(f16(phi) - f16(plo))
        perm = st["perm"]
        hpad = np.zeros((NC_NODES, 128), np.float32)
        hpad[:NL] = h[st["n0"]:st["n1"]]
        hpad = hpad[np.minimum(perm, NC_NODES - 1)] * (perm < NL)[:, None]
        vs = bf(hpad) @ bf(Wbig[:128]) + meanA @ f16(Wbig[128:])
        if use_wb3:
            hase = (np.where(perm < NL, deg[np.minimum(perm, NC_NODES - 1)], 0)
                    > 0).astype(np.float32)
            vs = vs + hase[:, None] * f16(wb3)[None, :]
        v, sc = vs[:, :256], vs[:, 256:]
        w = np.exp(sc)
        bl_o = np.full(NC_NODES, -1, np.int64)
        bl_o[:NL] = batch[st["n0"]:st["n1"]] - GL * c
        bl = np.where(perm < NL, bl_o[np.minimum(perm, NC_NODES - 1)], -1)
        onehot = (bl[:, None] == np.arange(GL)[None, :]).astype(np.float32)
        wv4 = np.concatenate([bf(w[:, :, None] * v.reshape(-1, 4, 64)).reshape(-1, 256),
                              bf(w)], 1)
        U = bf(onehot).T @ wv4
        den = np.maximum(U[:, 256:], 1e-30)
        pooled = U[:, :256].reshape(GL, 4, 64) / den[:, :, None]
        o = pooled.reshape(GL, 256) @ out_w.T + ob_eff
        outs.append(o)
    return np.concatenate(outs).reshape(G_TOTAL, 1, 256)


# ------------------------------------------------------------- bass program
def _build_program(cfg, use_wb3, debug=False):
    import concourse.bacc as bacc
    import concourse.mybir as mybir
    import concourse.tile as tile
    from bass_rust import InstructionNameOrderedSet

    def _nameset(names):
        s = InstructionNameOrderedSet()
        for n in names:
            s.add(n)
        return s

    F32 = mybir.dt.float32
    BF16 = mybir.dt.bfloat16
    F16 = mybir.dt.float16
    FP8 = mybir.dt.float8e4
    I16 = mybir.dt.int16
    AF = mybir.ActivationFunctionType
    B_S, G0, NGROUPS, E_PAD = cfg["B_S"], cfg["G0"], cfg["NGROUPS"], cfg["E_PAD"]
    IDX_COLS = sum(KB_LIST[s] * 8 for s in range(S_BATCH))

    nc = bacc.Bacc("TRN2", num_devices=CORES, num_swdge_queues=4)
    es_d = nc.dram_tensor("es", [E_PAD, 64], FP8, kind="ExternalInput")
    h_d = nc.dram_tensor("h", [P, NKB, 128], BF16, kind="ExternalInput")
    mem_d = nc.dram_tensor("mem", [P, NKB, GL], BF16, kind="ExternalInput")
    msk_d = nc.dram_tensor("msk", [64, NC_NODES], F16, kind="ExternalInput")
    idx_d = nc.dram_tensor("idx", [P, IDX_COLS], I16, kind="ExternalInput")
    tri_d = nc.dram_tensor("tri", [P, SPC], FP8, kind="ExternalInput")
    idtf_d = nc.dram_tensor("idtf", [GL, GL], F32, kind="ExternalInput")
    wb1_d = nc.dram_tensor("wb1", [128, 260], BF16, kind="ExternalInput")
    wb2_d = nc.dram_tensor("wb2", [64, 260], F16, kind="ExternalInput")
    owt_d = nc.dram_tensor("owt", [256, 256], F32, kind="ExternalInput")
    ob_d = nc.dram_tensor("ob", [GL, 256], F32, kind="ExternalInput")
    if use_wb3:
        wb3_d = nc.dram_tensor("wb3", [1, 260], F16, kind="ExternalInput")
        cst_d = nc.dram_tensor("cst", [1, NC_NODES], F16, kind="ExternalInput")
    y_d = nc.dram_tensor("y", [GL, 256], F32, kind="ExternalOutput")
    if debug:
        tabout_d = nc.dram_tensor("tabout", [P, NGROUPS * 8, 2, 64], F16,
                                  kind="ExternalOutput")
        amout_d = nc.dram_tensor("amout", [S_BATCH, 64, 8 * P], F16,
                                 kind="ExternalOutput")

    with tile.TileContext(nc) as tc:
        with tc.tile_pool(name="const", bufs=1) as cp, \
             tc.tile_pool(name="sb", bufs=3) as sb, \
             tc.tile_pool(name="big", bufs=1) as bigp, \
             tc.tile_pool(name="ps", bufs=2, space="PSUM") as ps, \
             tc.tile_pool(name="pacc", bufs=1, space="PSUM") as pacc:

            trib = cp.tile([P, SPC], FP8, name="trib")
            nc.sync.dma_start(out=trib[:], in_=tri_d.ap()[:, :])
            idtf = cp.tile([GL, GL], F32, name="idtf")
            nc.sync.dma_start(out=idtf[:], in_=idtf_d.ap()[:, :])
            wb1 = cp.tile([128, 260], BF16, name="wb1")
            nc.sync.dma_start(out=wb1[:], in_=wb1_d.ap()[:, :])
            wb2 = cp.tile([64, 260], F16, name="wb2")
            nc.sync.dma_start(out=wb2[:], in_=wb2_d.ap()[:, :])
            owt = cp.tile([P, 2, 256], F32, name="owt")
            nc.sync.dma_start(out=owt[:], in_=owt_d.ap()[:, :].rearrange("(i p) f -> p i f", p=P))
            obt = cp.tile([GL, 256], F32, name="obt")
            nc.sync.dma_start(out=obt[:], in_=ob_d.ap()[:, :])
            idxt = cp.tile([P, IDX_COLS], I16, name="idxt")
            nc.sync.dma_start(out=idxt[:], in_=idx_d.ap()[:, :])
            mskt = cp.tile([64, NC_NODES], F16, name="mskt")
            nc.sync.dma_start(out=mskt[:], in_=msk_d.ap()[:, :])
            if use_wb3:
                wb3 = cp.tile([1, 260], F16, name="wb3")
                nc.sync.dma_start(out=wb3[:], in_=wb3_d.ap()[:, :])
                cst = cp.tile([1, NC_NODES], F16, name="cst")
                nc.sync.dma_start(out=cst[:], in_=cst_d.ap()[:, :])

            # h pre-transposed: partition = feature, free = (chunk, node);
            # big loads go on the scalar HWDGE queue, off the edge stream.
            hsb = bigp.tile([P, NKB, 128], BF16, name="hsb")
            nc.scalar.dma_start(out=hsb[:], in_=h_d.ap()[:, :, :])
            memall = bigp.tile([P, NKB, GL], BF16, name="memall")
            nc.scalar.dma_start(out=memall[:], in_=mem_d.ap()[:, :, :])

            # fp16 prefix table, SBUF-resident, rows duplicated to 256B for
            # the transposing gather: row ridx -> partition ridx%128, rank
            # ridx//128 (= [group*8 + chunk%8] majors).
            table = bigp.tile([P, NGROUPS * 8, 2, 64], F16, name="table")

            pool_ps = pacc.tile([GL, 260], F32, name="pool_ps")

            xg_tiles = {}
            am_tiles = {}
            idx_off = [0]
            for s in range(S_BATCH):
                idx_off.append(idx_off[-1] + KB_LIST[s] * 8)

            # Transposing gathers stream elements through the per-core XBAR:
            # concurrent transpose gathers on different SWDGE queues cross-
            # pair their TX/RX descriptor streams and fetch wrong rows.  So:
            # all 8 preps go out up front (desc-gen in parallel on the 4
            # queue pairs, table reads deferred to trigger time), and the
            # triggers fire one at a time, ordered by a manual semaphore
            # chain (prep done + table casts done + previous gather done).
            dma_qsem = [nc.alloc_semaphore(f"gdma{q}") for q in range(4)]
            taba_sem = nc.alloc_semaphore("taba")
            tabd_sem = nc.alloc_semaphore("tabd")
            prot = {"last": None}
            last_cast = {"d": None, "a": None}

            def emit_prep(s):
                KBB = KB_LIST[s]
                nidx = KBB * P
                xg = sb.tile([P, 1, 8 * P], F16, name=f"xg{s}", tag="xg",
                             bufs=S_BATCH)
                p = nc.gpsimd.dma_gather(
                    out_ap=xg[:, :, :nidx],
                    in_ap=table[:, G0[s] * 8:G0[s + 1] * 8, :, :],
                    idxs_ap=idxt[:, idx_off[s]:idx_off[s + 1]],
                    num_idxs=nidx, num_idxs_reg=nidx, elem_size=128,
                    transpose=True, single_packet=False, queue_num=s % 4,
                    sbuf_tokens_per_rank=128,
                    sbuf_free_dim_per_rank=256,
                    prepare_only=True, sem=dma_qsem[s % 4],
                )
                xg_tiles[s] = xg
                return p

            for s in range(min(4, S_BATCH)):
                emit_prep(s)

            def emit_trigger_am(s):
                KBB = KB_LIST[s]
                nidx = KBB * P
                # one transpose gather in flight at a time (XBAR hazard);
                # desc-gen (the prep) runs ahead on the 4 queue pairs
                # markers retire after each engine's casts for this batch;
                # the trigger waits on the marker counts (the casts themselves
                # can't carry extra sem updates)
                for key, sem_, eng in (("d", tabd_sem, nc.vector),
                                       ("a", taba_sem, nc.scalar)):
                    mk = eng.wait_ge(sem_, 0).then_inc(sem_, 1)
                    if last_cast[key] is not None:
                        mk.ins.add_nosync_dependencies_from(
                            _nameset([last_cast[key].ins.name]))
                w1 = nc.gpsimd.wait_ge(taba_sem, s + 1)
                w1._wait_ge(tabd_sem, s + 1)
                if prot["last"] is not None:
                    w1.ins.add_nosync_dependencies_from(_nameset([prot["last"]]))
                wl = w1
                if s > 0:
                    w2 = nc.gpsimd.wait_ge(dma_qsem[(s - 1) % 4],
                                           16 * ((s - 1) // 4 + 1))
                    w2.ins.add_nosync_dependencies_from(_nameset([w1.ins.name]))
                    wl = w2
                trig = nc.gpsimd.trigger_dma(count=None, queue_num=s % 4)
                trig.ins.add_nosync_dependencies_from(_nameset([wl.ins.name]))
                prot["last"] = trig.ins.name
                if s + 4 < S_BATCH:
                    p = emit_prep(s + 4)
                    p.ins.add_nosync_dependencies_from(_nameset([trig.ins.name]))
                # am = phi - shift(phi)*mask on Pool: keeps the completion
                # wait off the DVE/ACT queues (they still have casts to run)
                xg = xg_tiles[s]
                n0 = KB0[s] * P
                am = sb.tile([64, 8 * P], F16, name=f"am{s}", tag="am",
                             bufs=S_BATCH)
                tmp = sb.tile([64, 8 * P], F16, name="tmp", tag="amt", bufs=2)
                w3 = nc.gpsimd.wait_ge(dma_qsem[s % 4], 16 * (s // 4 + 1))
                w3.ins.add_nosync_dependencies_from(_nameset([prot["last"]]))
                tt = nc.gpsimd.tensor_tensor(
                    out=tmp[:, :nidx - 1],
                    in0=xg[0:64, 0, 0:nidx - 1],
                    in1=mskt[:, n0 + 1:n0 + nidx],
                    op=mybir.AluOpType.mult)
                tt.ins.add_nosync_dependencies_from(_nameset([w3.ins.name]))
                nc.gpsimd.tensor_sub(out=am[:, 1:nidx],
                                     in0=xg[0:64, 0, 1:nidx],
                                     in1=tmp[:, :nidx - 1])
                cp0 = nc.gpsimd.tensor_copy(out=am[:, 0:1], in_=xg[0:64, 0, 0:1])
                cp0.ins.add_nosync_dependencies_from(_nameset([w3.ins.name]))
                if debug:
                    nc.sync.dma_start(out=amout_d.ap()[s, :, :nidx],
                                      in_=am[:, :nidx])
                am_tiles[s] = am

            def emit_dense(s):
                k0, KBB = KB0[s], KB_LIST[s]
                am = am_tiles[s]
                for k in range(k0, k0 + KBB):
                    j = k - k0
                    vs = ps.tile([P, 260], F32, name="vs", tag="vs", bufs=3)
                    nc.tensor.matmul(out=vs[:], lhsT=hsb[:, k, :], rhs=wb1[:],
                                     start=True, stop=False)
                    nc.tensor.matmul(out=vs[:], lhsT=am[:, j * P:(j + 1) * P],
                                     rhs=wb2[:], start=False, stop=not use_wb3)
                    if use_wb3:
                        nc.tensor.matmul(out=vs[:], lhsT=cst[:, k * P:(k + 1) * P],
                                         rhs=wb3[:], start=False, stop=True)
                    wsb = sb.tile([P, 4], F32, name="wsb", tag="wsb", bufs=4)
                    nc.scalar.activation(out=wsb[:], in_=vs[:, 256:260], func=AF.Exp)
                    pr = sb.tile([P, 260], BF16, name="pr", tag="pr", bufs=4)
                    nc.vector.tensor_tensor(
                        out=pr[:, :128].rearrange("p (h f) -> p h f", h=2),
                        in0=vs[:, :128].rearrange("p (h f) -> p h f", h=2),
                        in1=wsb[:, 0:2].broadcast_to([P, 2, 64]),
                        op=mybir.AluOpType.mult)
                    nc.scalar.activation(out=pr[:, 128:192], in_=vs[:, 128:192],
                                         func=AF.Copy, scale=wsb[:, 2:3])
                    nc.scalar.activation(out=pr[:, 192:256], in_=vs[:, 192:256],
                                         func=AF.Copy, scale=wsb[:, 3:4])
                    nc.scalar.copy(out=pr[:, 256:260], in_=wsb[:])
                    nc.tensor.matmul(out=pool_ps[:], lhsT=memall[:, k, :], rhs=pr[:],
                                     start=(k == 0), stop=(k == NKB - 1))

            import bisect
            for g in range(NGROUPS):
                et = sb.tile([P, CPG, 64], FP8, name="et", tag="et", bufs=6)
                nc.sync.dma_start(
                    out=et[:],
                    in_=es_d.ap()[g * GROUP_E:(g + 1) * GROUP_E, :]
                        .rearrange("(p c) f -> p c f", p=P))
                pp = ps.tile([P, 512], F32, name="pp", tag="pp", bufs=2)
                for m in range(4):
                    nc.tensor.matmul(
                        out=pp[32 * m:32 * m + 32, :],
                        lhsT=trib[:],
                        rhs=et[:, 8 * m:8 * m + 8, :].rearrange("p c f -> p (c f)"),
                        start=True, stop=True,
                        tile_position=(0, 32 * m))
                src = pp[:].rearrange("p (c f) -> p c f", c=8) \
                           .unsqueeze(2).broadcast_to([P, 8, 2, 64])
                if g % 2 == 0:
                    last_cast["d"] = nc.vector.tensor_copy(
                        out=table[:, g * 8:(g + 1) * 8, :, :], in_=src)
                else:
                    last_cast["a"] = nc.scalar.copy(
                        out=table[:, g * 8:(g + 1) * 8, :, :], in_=src)
                s = bisect.bisect_right(G0, g) - 1
                if g == G0[s + 1] - 1:
                    emit_trigger_am(s)
                    if s >= LAG:
                        emit_dense(s - LAG)

            for s in range(max(0, S_BATCH - LAG), S_BATCH):
                emit_dense(s)
            if debug:
                nc.sync.dma_start(out=tabout_d.ap()[:, :, :, :], in_=table[:])

            # ---- final: normalize + output projection
            den = sb.tile([GL, 4], F32, name="den")
            nc.vector.tensor_scalar_max(out=den[:], in0=pool_ps[:, 256:260],
                                        scalar1=1e-30)
            rden = sb.tile([GL, 4], F32, name="rden")
            nc.vector.reciprocal(out=rden[:], in_=den[:])
            pn = sb.tile([GL, 256], F32, name="pn")
            for hh in range(NUM_HEADS):
                nc.vector.tensor_scalar_mul(out=pn[:, 64 * hh:64 * hh + 64],
                                            in0=pool_ps[:, 64 * hh:64 * hh + 64],
                                            scalar1=rden[:, hh:hh + 1])
            pnT = sb.tile([P, 2, GL], F32, name="pnT")
            for i in range(2):
                ptp = ps.tile([P, GL], F32, name="ptp", tag="vs", bufs=3)
                nc.tensor.transpose(out=ptp[:], in_=pn[:, i * P:(i + 1) * P],
                                    identity=idtf[:])
                nc.vector.tensor_copy(out=pnT[:, i, :], in_=ptp[:])
            ops_t = ps.tile([GL, 256], F32, name="ops_t", tag="vs", bufs=3)
            for i in range(2):
                nc.tensor.matmul(out=ops_t[:], lhsT=pnT[:, i, :], rhs=owt[:, i, :],
                                 start=(i == 0), stop=(i == 1))
            osb = sb.tile([GL, 256], F32, name="osb")
            nc.vector.tensor_add(out=osb[:], in0=ops_t[:], in1=obt[:])
            nc.sync.dma_start(out=y_d.ap()[:, :], in_=osb[:])

    nc.finalize()
    return nc


_CACHE = {}


def _get_program(cfg, use_wb3):
    key = (tuple(cfg["B_S"]), use_wb3)
    if key not in _CACHE:
        _CACHE[key] = _build_program(cfg, use_wb3)
    return _CACHE[key]


def kernel(h, edge_index, edge_attr, batch, num_graphs,
           node_w, node_b, edge_w, edge_b, query, in_w, in_b, out_w, out_b,
           _trace=False, _numpy_only=False):
    import ml_dtypes
    h = np.asarray(h, np.float32)
    edge_attr = np.asarray(edge_attr, np.float32)
    batch_np = np.asarray(batch, np.int64)
    assert int(num_graphs) == G_TOTAL

    cores, cfg = _prep(h, edge_index, batch_np)
    Wbig, ob_eff, wb3 = _fold_weights(
        np.asarray(node_w, np.float32), np.asarray(node_b, np.float32),
        np.asarray(edge_w, np.float32), np.asarray(edge_b, np.float32),
        np.asarray(query, np.float32), np.asarray(in_w, np.float32),
        np.asarray(in_b, np.float32), np.asarray(out_w, np.float32),
        np.asarray(out_b, np.float32))
    use_wb3 = bool(np.any(np.asarray(edge_b, np.float32) != 0))
    if _numpy_only:
        return _numpy_device_model(cores, cfg, h, edge_attr, batch_np, Wbig,
                                   wb3, use_wb3, ob_eff,
                                   np.asarray(out_w, np.float32),
                                   np.asarray(out_b, np.float32))

    in_maps = _build_in_maps(cores, cfg, h, edge_attr, batch_np, Wbig,
                             ob_eff, wb3, use_wb3,
                             np.asarray(out_w, np.float32))
    from concourse.bass_utils import run_bass_kernel_spmd
    nc = _get_program(cfg, use_wb3)
    res = run_bass_kernel_spmd(nc, in_maps, core_ids=list(range(CORES)),
                               trace=_trace)
    out = np.concatenate([np.asarray(res.results[c]["y"], np.float32)
                          for c in range(CORES)])
    kernel._last_result = res
    return out.reshape(G_TOTAL, 1, 256)


def _build_in_maps(cores, cfg, h, edge_attr, batch_np, Wbig, ob_eff, wb3,
                   use_wb3, out_w):
    import ml_dtypes
    bf16 = ml_dtypes.bfloat16
    fp8 = ml_dtypes.float8_e4m3fn
    tri = (np.arange(P)[:, None] // SLOT <= np.arange(SPC)[None, :]).astype(np.float32)
    shared = dict(
        tri=tri.astype(fp8),
        idtf=np.eye(GL, dtype=np.float32),
        wb1=Wbig[:128].astype(bf16), wb2=Wbig[128:].astype(np.float16),
        owt=np.ascontiguousarray(out_w.T),
        ob=np.tile(ob_eff[None, :], (GL, 1)).astype(np.float32),
    )
    if use_wb3:
        shared["wb3"] = wb3[None, :].astype(np.float16)
    in_maps = []
    for c, st in enumerate(cores):
        NL = st["NL"]
        deg = np.pad(st["deg"], (0, NC_NODES - NL)).astype(np.float32)
        inv = 1.0 / np.maximum(deg, 1.0)
        stream = np.zeros((cfg["E_PAD"], 64), np.float32)
        stream[st["epos"]] = edge_attr[st["eord"]] * inv[st["lrow"]][:, None]
        perm = st["perm"]
        real = (perm < NL)
        safe = np.minimum(perm, NC_NODES - 1)
        hpad = np.zeros((NC_NODES, 128), np.float32)
        hpad[:NL] = h[st["n0"]:st["n1"]]
        hpad = hpad[safe] * real[:, None]
        h3 = np.ascontiguousarray(hpad.reshape(-1, P, 128).transpose(2, 0, 1))
        bl_o = np.full(NC_NODES, -1, np.int64)
        bl_o[:NL] = batch_np[st["n0"]:st["n1"]] - GL * c
        bl = np.where(real, bl_o[safe], -1)
        mem = (bl.reshape(NKB, P)[:, :, None] == np.arange(GL)[None, None, :])
        mem3 = np.ascontiguousarray(mem.transpose(1, 0, 2)).astype(bf16)
        idx = _wrap_idx(st["hi_row"])
        im = dict(es=stream.astype(fp8), h=h3.astype(bf16), mem=mem3,
                  idx=idx,
                  msk=np.ascontiguousarray(np.broadcast_to(
                      st["mask"].astype(np.float16)[None, :], (64, NC_NODES))),
                  **shared)
        if use_wb3:
            im["cst"] = np.where(real, (deg[safe] > 0), False) \
                          .astype(np.float16)[None, :]
        in_maps.append(im)
    return in_maps
